# revision 11
# baseline (speedup 1.0000x reference)
import sys

sys.path.insert(0, "/opt/trn_rl_repo")

from contextlib import ExitStack

import numpy as np

import concourse.bacc as bacc
import concourse.bass as bass
import concourse.mybir as mybir
import concourse.tile as tile
from concourse import bass_utils
from concourse._compat import get_trn_type

# Problem constants (nn_KAN_layer: IN=OUT=64, HIDDEN=[64,64,64], B=256, D=4096)
B = 256
IN_DIM = 64
OUT_DIM = 64
H = 64
D = IN_DIM * OUT_DIM
NCORES = 8
OPC = OUT_DIM // NCORES  # o values per core = 8
NPAIR = OPC // 2 * IN_DIM  # pairs per core = 256

F32 = mybir.dt.float32

_PROGRAM_CACHE = {}


def _build_program():
    """Single-core Bass program, run SPMD on 8 cores with per-core data.

    Per core: 512 edges (o in [8c, 8c+8), all i). Pair p = (i=p//4, q=p%4)
    covers edges A=(8c+2q, i), B=(8c+2q+1, i).
      L0: K=2 matmul (w0 row + bias row vs x row + ones row) -> psum [128,256]
      L1/L2: two 64x64 matmuls in opposite PE quadrants -> psum [128,256]
      Lout: K=128, M=8 matmul accumulating into a persistent psum [8,256]
    Relu drains alternate between ScalarE (activation w/ bias) and VectorE
    (tensor_scalar add-bias then max-0).
    """
    nc = bacc.Bacc(
        get_trn_type() or "TRN2",
        target_bir_lowering=False,
        debug=False,
        num_devices=1,
    )

    xw_d = nc.dram_tensor("xw", [32, 4 * B], F32, kind="ExternalInput").ap()
    w0b0_d = nc.dram_tensor("w0b0", [NPAIR, 32, 128], F32, kind="ExternalInput").ap()
    w1c_d = nc.dram_tensor("w1c", [NPAIR, 128, H], F32, kind="ExternalInput").ap()
    w2c_d = nc.dram_tensor("w2c", [NPAIR, 128, H], F32, kind="ExternalInput").ap()
    bc_d = nc.dram_tensor("bc", [128, 2 * NPAIR], F32, kind="ExternalInput").ap()
    woutc_d = nc.dram_tensor("woutc", [128, 8 * NPAIR], F32, kind="ExternalInput").ap()
    boutc_d = nc.dram_tensor("boutc", [OPC, 1], F32, kind="ExternalInput").ap()
    y_d = nc.dram_tensor("y", [OPC, B], F32, kind="ExternalOutput").ap()

    with tile.TileContext(nc) as tc, ExitStack() as ctx:
        const = ctx.enter_context(tc.tile_pool(name="const", bufs=1))
        xw_s = const.tile([32, 4 * B], F32)
        nc.sync.dma_start(xw_s[:], xw_d)
        bc_s = const.tile([128, 2 * NPAIR], F32)
        nc.sync.dma_start(bc_s[:], bc_d)
        woutc_s = const.tile([128, 8 * NPAIR], F32)
        nc.sync.dma_start(woutc_s[:], woutc_d)
        bout_s = const.tile([OPC, 1], F32)
        nc.sync.dma_start(bout_s[:], boutc_d)
        zero_s = const.tile([128, 1], F32)
        nc.any.memset(zero_s[:], 0.0)

        wpool = ctx.enter_context(tc.tile_pool(name="w", bufs=8))
        apool = ctx.enter_context(tc.tile_pool(name="act", bufs=8))
        zpool = ctx.enter_context(tc.tile_pool(name="z", bufs=6, space="PSUM"))
        ypool = ctx.enter_context(tc.tile_pool(name="y", bufs=1, space="PSUM"))
        ypsum = ypool.tile([OPC, B], F32)

        drain_ctr = [0]

        def relu_drain(dst, src, bias_ap):
            if drain_ctr[0] % 2 == 0:
                nc.scalar.activation(
                    dst, src, mybir.ActivationFunctionType.Relu, bias=bias_ap
                )
            else:
                nc.vector.tensor_scalar(
                    dst,
                    src,
                    scalar1=bias_ap,
                    scalar2=0.0,
                    op0=mybir.AluOpType.add,
                    op1=mybir.AluOpType.max,
                )
            drain_ctr[0] += 1

        for p in range(NPAIR):
            i, q = p // 4, p % 4
            g = i // 16  # i-group: xw free-dim slice
            # L0: z0 = w0_pair x_i^T + b0_pair (one-hot K=32 lhsT, bias via ones row)
            w0t = wpool.tile([32, 128], F32, tag="w0")
            nc.sync.dma_start(w0t[:], w0b0_d[p])
            z0 = zpool.tile([128, B], F32, tag="z")
            nc.tensor.matmul(
                z0[:],
                w0t[:],
                xw_s[:, B * g : B * g + B],
                start=True,
                stop=True,
            )
            a1 = apool.tile([128, B], F32, tag="a")
            relu_drain(a1[:], z0[:], zero_s[:])

            # L1: per-edge 64x64 matmuls in opposite quadrants
            w1t = wpool.tile([128, H], F32, tag="w")
            nc.sync.dma_start(w1t[:], w1c_d[p])
            z1 = zpool.tile([128, B], F32, tag="z")
            nc.tensor.matmul(
                z1[0:64, :], w1t[0:64, :], a1[0:64, :], start=True, stop=True,
                tile_position=(0, 0),
            )
            nc.tensor.matmul(
                z1[64:128, :], w1t[64:128, :], a1[64:128, :], start=True, stop=True,
                tile_position=(64, 64),
            )
            a2 = apool.tile([128, B], F32, tag="a")
            relu_drain(a2[:], z1[:], bc_s[:, 2 * p : 2 * p + 1])

            # L2
            w2t = wpool.tile([128, H], F32, tag="w")
            nc.sync.dma_start(w2t[:], w2c_d[p])
            z2 = zpool.tile([128, B], F32, tag="z")
            nc.tensor.matmul(
                z2[0:64, :], w2t[0:64, :], a2[0:64, :], start=True, stop=True,
                tile_position=(0, 0),
            )
            nc.tensor.matmul(
                z2[64:128, :], w2t[64:128, :], a2[64:128, :], start=True, stop=True,
                tile_position=(64, 64),
            )
            a3 = apool.tile([128, B], F32, tag="a")
            relu_drain(a3[:], z2[:], bc_s[:, 2 * p + 1 : 2 * p + 2])

            # Lout: accumulate y[o_local, b] += wout_pair^T @ a3
            nc.tensor.matmul(
                ypsum[:],
                woutc_s[:, 8 * p : 8 * p + 8],
                a3[:],
                start=(p == 0),
                stop=(p == NPAIR - 1),
            )

        y_s = apool.tile([OPC, B], F32, tag="y")
        nc.vector.tensor_scalar(
            y_s[:], ypsum[:], scalar1=bout_s[:], scalar2=None,
            op0=mybir.AluOpType.add,
        )
        nc.sync.dma_start(y_d, y_s[:])

    nc.compile()
    return nc


def _pack_inputs(x, w0, b0, w1, b1, w2, b2, wout, bout):
    """Host-side packing into per-core SPMD inputs."""
    f = np.float32
    x = np.asarray(x, f)
    w0 = np.asarray(w0, f)
    b0 = np.asarray(b0, f)
    w1 = np.asarray(w1, f)
    b1 = np.asarray(b1, f)
    w2 = np.asarray(w2, f)
    b2 = np.asarray(b2, f)
    wout = np.asarray(wout, f)
    bout = np.asarray(bout, f)

    # xw[2j, B*g:B*(g+1)] = x[:, 16g+j]; xw[2j+1, :] = 1 (ones row for bias fold)
    xw = np.empty((32, 4 * B), f)
    for g in range(4):
        xw[0::2, B * g : B * (g + 1)] = x.T[16 * g : 16 * (g + 1)]
    xw[1::2] = 1.0
    woutr = wout.reshape(OUT_DIM, IN_DIM, H)

    i_idx = np.repeat(np.arange(IN_DIM), 4)  # p = 4*i + q
    q_idx = np.tile(np.arange(4), IN_DIM)
    P = np.arange(NPAIR)

    in_maps = []
    for c in range(NCORES):
        oA = 8 * c + 2 * q_idx
        oB = oA + 1
        dA = oA * IN_DIM + i_idx
        dB = oB * IN_DIM + i_idx

        # w0b0[p]: [32, 128] one-hot lhsT; row 2j = w0 pair concat, row 2j+1 = b0
        j_idx = i_idx % 16
        w0b0 = np.zeros((NPAIR, 32, 128), f)
        w0b0[P, 2 * j_idx, :64] = w0[oA, i_idx]
        w0b0[P, 2 * j_idx, 64:] = w0[oB, i_idx]
        w0b0[P, 2 * j_idx + 1, :64] = b0[dA]
        w0b0[P, 2 * j_idx + 1, 64:] = b0[dB]

        w1c = np.empty((NPAIR, 128, H), f)
        w1c[:, :64, :] = w1[dA].transpose(0, 2, 1)
        w1c[:, 64:, :] = w1[dB].transpose(0, 2, 1)
        w2c = np.empty((NPAIR, 128, H), f)
        w2c[:, :64, :] = w2[dA].transpose(0, 2, 1)
        w2c[:, 64:, :] = w2[dB].transpose(0, 2, 1)

        BC = np.empty((2, H, NPAIR, 2), f)
        BC[0, :, :, 0] = b1[dA].T
        BC[1, :, :, 0] = b1[dB].T
        BC[0, :, :, 1] = b2[dA].T
        BC[1, :, :, 1] = b2[dB].T
        bc = np.ascontiguousarray(BC.reshape(128, 2 * NPAIR))

        WC = np.zeros((2, H, NPAIR, OPC), f)
        WC[0, :, P, 2 * q_idx] = woutr[oA, i_idx]
        WC[1, :, P, 2 * q_idx + 1] = woutr[oB, i_idx]
        woutc = np.ascontiguousarray(WC.reshape(128, 8 * NPAIR))

        boutc = np.ascontiguousarray(bout[8 * c : 8 * c + 8, None])

        in_maps.append(
            {
                "xw": xw,
                "w0b0": w0b0,
                "w1c": w1c,
                "w2c": w2c,
                "bc": bc,
                "woutc": woutc,
                "boutc": boutc,
            }
        )
    return in_maps


def _get_program():
    if "nc" not in _PROGRAM_CACHE:
        _PROGRAM_CACHE["nc"] = _build_program()
    return _PROGRAM_CACHE["nc"]


def kernel(x, w0, b0, w1, b1, w2, b2, wout, bout, _trace=False, _trace_kwargs=None):
    nc = _get_program()
    in_maps = _pack_inputs(x, w0, b0, w1, b1, w2, b2, wout, bout)
    res = bass_utils.run_bass_kernel_spmd(
        nc,
        in_maps,
        core_ids=list(range(NCORES)),
        trace=_trace,
        **(_trace_kwargs or {}),
    )
    kernel.last_results = res
    ys = [res.results[c]["y"] for c in range(NCORES)]
    yfull = np.concatenate(ys, axis=0)  # [64, 256]
    return np.ascontiguousarray(yfull.T).astype(np.float32)


# revision 12
# speedup vs baseline: 3.2891x; 3.2891x over previous
import sys

sys.path.insert(0, "/opt/trn_rl_repo")

from contextlib import ExitStack

import ml_dtypes
import numpy as np

import concourse.bacc as bacc
import concourse.bass as bass
import concourse.mybir as mybir
import concourse.tile as tile
from concourse import bass_utils
from concourse._compat import get_trn_type

# Problem constants (nn_KAN_layer: IN=OUT=64, HIDDEN=[64,64,64], B=256, D=4096)
B = 256
IN_DIM = 64
OUT_DIM = 64
H = 64
D = IN_DIM * OUT_DIM
NCORES = 8
OPC = OUT_DIM // NCORES  # o values per core = 8
NPAIR = OPC // 2 * IN_DIM  # pairs per core = 256
NT = NPAIR // 2  # 2-pair groups = 128

F32 = mybir.dt.float32
BF16 = mybir.dt.bfloat16
NPBF = ml_dtypes.bfloat16

_PROGRAM_CACHE = {}


def _build_program():
    """Single-core Bass program, run SPMD on 8 cores with per-core data.

    Per core: 512 edges (o in [8c, 8c+8), all i). Pair p = (i=p//4, q=p%4)
    covers edges A=(8c+2q, i), B=(8c+2q+1, i). Groups t = (pairs 2t, 2t+1)
    share the same i.
      L0: one-hot K=32 bf16 matmuls (bias folded via ones row) for both
          pairs into one [128,512] psum bank; single FD=512 pure-relu drain.
      L1/L2: per-edge 64x64 bf16 matmuls in opposite PE quadrants; FD=256
          drain w/ per-partition bias, alternating ScalarE/VectorE.
      Lout: K=128, M=8 bf16 matmul accumulating into persistent psum [8,256].
    """
    nc = bacc.Bacc(
        get_trn_type() or "TRN2",
        target_bir_lowering=False,
        debug=False,
        num_devices=1,
    )

    xw_d = nc.dram_tensor("xw", [32, 4 * B], BF16, kind="ExternalInput").ap()
    w0b0_d = nc.dram_tensor("w0b0", [NT, 32, 256], BF16, kind="ExternalInput").ap()
    w1c_d = nc.dram_tensor("w1c", [NT, 128, 128], BF16, kind="ExternalInput").ap()
    w2c_d = nc.dram_tensor("w2c", [NT, 128, 128], BF16, kind="ExternalInput").ap()
    bc_d = nc.dram_tensor("bc", [128, 2 * NPAIR], F32, kind="ExternalInput").ap()
    woutc_d = nc.dram_tensor("woutc", [128, 8 * NPAIR], BF16, kind="ExternalInput").ap()
    boutc_d = nc.dram_tensor("boutc", [OPC, 1], F32, kind="ExternalInput").ap()
    y_d = nc.dram_tensor("y", [OPC, B], F32, kind="ExternalOutput").ap()

    with tile.TileContext(nc) as tc, ExitStack() as ctx:
        const = ctx.enter_context(tc.tile_pool(name="const", bufs=1))
        xw_s = const.tile([32, 4 * B], BF16)
        nc.sync.dma_start(xw_s[:], xw_d)
        bc_s = const.tile([128, 2 * NPAIR], F32)
        nc.sync.dma_start(bc_s[:], bc_d)
        woutc_s = const.tile([128, 8 * NPAIR], BF16)
        nc.sync.dma_start(woutc_s[:], woutc_d)
        bout_s = const.tile([OPC, 1], F32)
        nc.sync.dma_start(bout_s[:], boutc_d)
        zero_s = const.tile([128, 1], F32)
        nc.any.memset(zero_s[:], 0.0)

        wpool = ctx.enter_context(tc.tile_pool(name="w", bufs=6))
        apool = ctx.enter_context(tc.tile_pool(name="act", bufs=6))
        z0pool = ctx.enter_context(tc.tile_pool(name="z0", bufs=2, space="PSUM"))
        zpool = ctx.enter_context(tc.tile_pool(name="z", bufs=4, space="PSUM"))
        ypool = ctx.enter_context(tc.tile_pool(name="y", bufs=1, space="PSUM"))
        ypsum = ypool.tile([OPC, B], F32)

        drain_ctr = [0]

        def relu_drain(dst, src, bias_ap):
            if drain_ctr[0] % 2 == 0:
                if bias_ap is None:
                    nc.scalar.activation(
                        dst, src, mybir.ActivationFunctionType.Relu, bias=zero_s[:]
                    )
                else:
                    nc.scalar.activation(
                        dst, src, mybir.ActivationFunctionType.Relu, bias=bias_ap
                    )
            else:
                if bias_ap is None:
                    nc.vector.tensor_scalar(
                        dst, src, scalar1=0.0, scalar2=None, op0=mybir.AluOpType.max
                    )
                else:
                    nc.vector.tensor_scalar(
                        dst,
                        src,
                        scalar1=bias_ap,
                        scalar2=0.0,
                        op0=mybir.AluOpType.add,
                        op1=mybir.AluOpType.max,
                    )
            drain_ctr[0] += 1

        for t in range(NT):
            i = t // 2  # shared input column for both pairs in the group
            g = i // 16
            # L0: both pairs into one psum bank, single pure-relu drain
            w0t = wpool.tile([32, 256], BF16, tag="w0")
            nc.sync.dma_start(w0t[:], w0b0_d[t])
            z0 = z0pool.tile([128, 512], F32, tag="z0")
            nc.tensor.matmul(
                z0[:, 0:B], w0t[:, 0:128], xw_s[:, B * g : B * g + B],
                start=True, stop=True,
            )
            nc.tensor.matmul(
                z0[:, B : 2 * B], w0t[:, 128:256], xw_s[:, B * g : B * g + B],
                start=True, stop=True,
            )
            a1 = apool.tile([128, 512], BF16, tag="a1")
            relu_drain(a1[:], z0[:], None)

            w1t = wpool.tile([128, 128], BF16, tag="w1")
            nc.sync.dma_start(w1t[:], w1c_d[t])
            w2t = wpool.tile([128, 128], BF16, tag="w2")
            nc.sync.dma_start(w2t[:], w2c_d[t])

            for s in (0, 1):
                p = 2 * t + s
                # L1
                z1 = zpool.tile([128, B], F32, tag="z")
                nc.tensor.matmul(
                    z1[0:64, :], w1t[0:64, 64 * s : 64 * s + 64],
                    a1[0:64, B * s : B * s + B],
                    start=True, stop=True, tile_position=(0, 0),
                )
                nc.tensor.matmul(
                    z1[64:128, :], w1t[64:128, 64 * s : 64 * s + 64],
                    a1[64:128, B * s : B * s + B],
                    start=True, stop=True, tile_position=(64, 64),
                )
                a2 = apool.tile([128, B], BF16, tag="a2")
                relu_drain(a2[:], z1[:], bc_s[:, 2 * p : 2 * p + 1])

                # L2
                z2 = zpool.tile([128, B], F32, tag="z")
                nc.tensor.matmul(
                    z2[0:64, :], w2t[0:64, 64 * s : 64 * s + 64], a2[0:64, :],
                    start=True, stop=True, tile_position=(0, 0),
                )
                nc.tensor.matmul(
                    z2[64:128, :], w2t[64:128, 64 * s : 64 * s + 64], a2[64:128, :],
                    start=True, stop=True, tile_position=(64, 64),
                )
                a3 = apool.tile([128, B], BF16, tag="a3")
                relu_drain(a3[:], z2[:], bc_s[:, 2 * p + 1 : 2 * p + 2])

                # Lout: accumulate y[o_local, b] += wout_pair^T @ a3
                nc.tensor.matmul(
                    ypsum[:],
                    woutc_s[:, 8 * p : 8 * p + 8],
                    a3[:],
                    start=(p == 0),
                    stop=(p == NPAIR - 1),
                )

        y_s = apool.tile([OPC, B], F32, tag="y")
        nc.vector.tensor_scalar(
            y_s[:], ypsum[:], scalar1=bout_s[:], scalar2=None,
            op0=mybir.AluOpType.add,
        )
        nc.sync.dma_start(y_d, y_s[:])

    nc.compile()
    return nc


def _pack_inputs(x, w0, b0, w1, b1, w2, b2, wout, bout):
    """Host-side packing into per-core SPMD inputs."""
    f = np.float32
    x = np.asarray(x, f)
    w0 = np.asarray(w0, f)
    b0 = np.asarray(b0, f)
    w1 = np.asarray(w1, f)
    b1 = np.asarray(b1, f)
    w2 = np.asarray(w2, f)
    b2 = np.asarray(b2, f)
    wout = np.asarray(wout, f)
    bout = np.asarray(bout, f)

    # xw[2j, B*g:B*(g+1)] = x[:, 16g+j]; xw[2j+1, :] = 1 (ones row for bias fold)
    xw = np.empty((32, 4 * B), f)
    for g in range(4):
        xw[0::2, B * g : B * (g + 1)] = x.T[16 * g : 16 * (g + 1)]
    xw[1::2] = 1.0
    xw = xw.astype(NPBF)
    woutr = wout.reshape(OUT_DIM, IN_DIM, H)

    i_idx = np.repeat(np.arange(IN_DIM), 4)  # p = 4*i + q
    q_idx = np.tile(np.arange(4), IN_DIM)
    P = np.arange(NPAIR)

    in_maps = []
    for c in range(NCORES):
        oA = 8 * c + 2 * q_idx
        oB = oA + 1
        dA = oA * IN_DIM + i_idx
        dB = oB * IN_DIM + i_idx

        # w0b0[p]: [32, 128] one-hot lhsT; row 2j = w0 pair concat, row 2j+1 = b0
        j_idx = i_idx % 16
        w0b0 = np.zeros((NPAIR, 32, 128), f)
        w0b0[P, 2 * j_idx, :64] = w0[oA, i_idx]
        w0b0[P, 2 * j_idx, 64:] = w0[oB, i_idx]
        w0b0[P, 2 * j_idx + 1, :64] = b0[dA]
        w0b0[P, 2 * j_idx + 1, 64:] = b0[dB]
        # group 2 pairs side by side: [NT, 32, 256]
        w0b0 = (
            w0b0.reshape(NT, 2, 32, 128)
            .transpose(0, 2, 1, 3)
            .reshape(NT, 32, 256)
            .astype(NPBF)
        )

        w1c = np.empty((NPAIR, 128, H), f)
        w1c[:, :64, :] = w1[dA].transpose(0, 2, 1)
        w1c[:, 64:, :] = w1[dB].transpose(0, 2, 1)
        w1c = (
            w1c.reshape(NT, 2, 128, H).transpose(0, 2, 1, 3).reshape(NT, 128, 128)
        ).astype(NPBF)
        w2c = np.empty((NPAIR, 128, H), f)
        w2c[:, :64, :] = w2[dA].transpose(0, 2, 1)
        w2c[:, 64:, :] = w2[dB].transpose(0, 2, 1)
        w2c = (
            w2c.reshape(NT, 2, 128, H).transpose(0, 2, 1, 3).reshape(NT, 128, 128)
        ).astype(NPBF)

        BC = np.empty((2, H, NPAIR, 2), f)
        BC[0, :, :, 0] = b1[dA].T
        BC[1, :, :, 0] = b1[dB].T
        BC[0, :, :, 1] = b2[dA].T
        BC[1, :, :, 1] = b2[dB].T
        bc = np.ascontiguousarray(BC.reshape(128, 2 * NPAIR))

        WC = np.zeros((2, H, NPAIR, OPC), f)
        WC[0, :, P, 2 * q_idx] = woutr[oA, i_idx]
        WC[1, :, P, 2 * q_idx + 1] = woutr[oB, i_idx]
        woutc = np.ascontiguousarray(WC.reshape(128, 8 * NPAIR)).astype(NPBF)

        boutc = np.ascontiguousarray(bout[8 * c : 8 * c + 8, None])

        in_maps.append(
            {
                "xw": xw,
                "w0b0": w0b0,
                "w1c": w1c,
                "w2c": w2c,
                "bc": bc,
                "woutc": woutc,
                "boutc": boutc,
            }
        )
    return in_maps


def _get_program():
    if "nc" not in _PROGRAM_CACHE:
        _PROGRAM_CACHE["nc"] = _build_program()
    return _PROGRAM_CACHE["nc"]


def kernel(x, w0, b0, w1, b1, w2, b2, wout, bout, _trace=False, _trace_kwargs=None):
    nc = _get_program()
    in_maps = _pack_inputs(x, w0, b0, w1, b1, w2, b2, wout, bout)
    res = bass_utils.run_bass_kernel_spmd(
        nc,
        in_maps,
        core_ids=list(range(NCORES)),
        trace=_trace,
        **(_trace_kwargs or {}),
    )
    kernel.last_results = res
    ys = [res.results[c]["y"] for c in range(NCORES)]
    yfull = np.concatenate(ys, axis=0)  # [64, 256]
    return np.ascontiguousarray(yfull.T).astype(np.float32)
